# revision 4
# baseline (speedup 1.0000x reference)
"""ConceptNet Trainium2 kernel: 8-core SPMD.

Sharding: sampled_train_embeddings column-sharded across 8 cores (each core
computes cx_dot / dist for its 12500 columns and extracts a local top-32 per
concept row); train_embedding batch-sharded (each core computes rec/prob for
1024 rows); concept / rec vectors replicated.  Host merges local top-k into
the global top-64 mean (L_sparse_1) and concatenates rec/prob shards.

Device algorithm notes:
 - KNN GEMM in bf16 (PE), x2 column norms via bf16 squares + PE ones-reduce.
 - Selection metric m' = cx - x2/2 + 512 folded directly into the PSUM
   accumulation (a K=1 matmul adds the per-column row w = 512 - x2/2), then
   stored bf16.  The +512 shift centers m' near 0 so bf16 quantization is
   ~5x finer (validated: L1 rel err 1.8e-4 vs 4e-3 unshifted).
 - Top-32 per row via 4 rounds of vector.max + max_index + match_replace.
 - score GEMM in true fp32 (threshold margin of score_n vs 0.1 is 1.3e-6;
   fp32r's 1.6e-4 error would flip entries).  MLP + gram in fp32r.
 - L_sparse_2 partials: off-diagonal (affine_select) gram squares, summed
   per partition on device; host sums 512 values.
"""

import os
import numpy as np
import ml_dtypes

import concourse.bass as bass
import concourse.bacc as bacc
import concourse.mybir as mybir
from concourse import tile
from concourse.bass_utils import run_bass_kernel_spmd
from concourse.masks import make_identity

F32 = mybir.dt.float32
F32R = mybir.dt.float32r
BF16 = mybir.dt.bfloat16
U32 = mybir.dt.uint32
AF = mybir.ActivationFunctionType
ALU = mybir.AluOpType
AX = mybir.AxisListType

D, C, N, B, H = 1024, 512, 100000, 8192, 1024
NCORES = 8
NS, BS = N // NCORES, B // NCORES           # 12500, 1024
NC = 500                                    # columns per chunk (one PSUM bank)
KT, CTN = D // 128, C // 128                # 8 k-tiles, 4 concept tiles
KL = 32                                     # local top-k per shard
SHIFT = 512.0
NEG = -3.0e38


def build(ns=NS, bs=BS, profile_dummy=False):
    nch = ns // NC
    btn = bs // 128
    nc = bacc.Bacc("TRN2", target_bir_lowering=False, debug=False,
                   num_devices=NCORES)

    te = nc.dram_tensor("te", [bs, D], F32, kind="ExternalInput").ap()
    xs = nc.dram_tensor("xs", [D, ns], F32, kind="ExternalInput").ap()
    con = nc.dram_tensor("con", [D, C], F32, kind="ExternalInput").ap()
    r1 = nc.dram_tensor("r1", [C, H], F32, kind="ExternalInput").ap()
    r2 = nc.dram_tensor("r2", [H, D], F32, kind="ExternalInput").ap()

    rec_o = nc.dram_tensor("rec", [bs, D], F32, kind="ExternalOutput").ap()
    prob_o = nc.dram_tensor("prob", [bs, C], F32, kind="ExternalOutput").ap()
    mtop_o = nc.dram_tensor("mtop", [C, KL], F32, kind="ExternalOutput").ap()
    itop_o = nc.dram_tensor("itop", [C, KL], U32, kind="ExternalOutput").ap()
    x2_o = nc.dram_tensor("x2o", [1, ns], F32, kind="ExternalOutput").ap()
    goff_o = nc.dram_tensor("goff", [128, CTN], F32, kind="ExternalOutput").ap()

    with tile.TileContext(nc) as tc:
        with tc.tile_pool(name="outer", bufs=1) as outer:
            # persistent small tensors
            conb = [outer.tile([128, C], BF16, tag=f"conb{k}", name=f"conb{k}") for k in range(KT)]
            ident = outer.tile([128, 128], F32, tag="ident")
            make_identity(nc, ident[:])
            ones_col_b = outer.tile([128, 1], BF16, tag="onescb")
            nc.vector.memset(ones_col_b[:], 1.0)
            ones_row_b = outer.tile([1, 128], BF16, tag="onesrb")
            nc.vector.memset(ones_row_b[:], 1.0)
            ones_col_f = outer.tile([128, 1], F32, tag="onescf")
            nc.vector.memset(ones_col_f[:], 1.0)
            ones_row_f = outer.tile([1, 128], F32, tag="onesrf")
            nc.vector.memset(ones_row_f[:], 1.0)
            nte = outer.tile([128, btn], F32, tag="nte")
            thr128 = outer.tile([128, C], F32, tag="thr128")
            # m metric buffers (freed after extraction via pool nesting below)
            with tc.tile_pool(name="mpool", bufs=1) as mpool:
                m = [mpool.tile([128, ns], BF16, tag=f"m{ct}", name=f"m{ct}") for ct in range(CTN)]

                # ---------------- KNN chunk loop ----------------
                with tc.tile_pool(name="kchunk", bufs=2) as kp, \
                     tc.tile_pool(name="kpsum", bufs=2, space="PSUM") as kps, \
                     tc.tile_pool(name="cpsum", bufs=1, space="PSUM") as cps:
                    # load + cast concept once (f32 tiles transient here)
                    for k in range(KT):
                        cf = kp.tile([128, C], F32, tag="conf_tmp")
                        nc.sync.dma_start(cf[:], con[k * 128:(k + 1) * 128, :])
                        nc.gpsimd.tensor_copy(out=conb[k][:], in_=cf[:])
                    for j in range(nch):
                        js = slice(j * NC, (j + 1) * NC)
                        xb = []
                        x2p = kps.tile([1, NC], F32, tag="x2p")
                        for k in range(KT):
                            xf = kp.tile([128, NC], F32, tag=f"xf{k}")
                            nc.sync.dma_start(xf[:], xs[k * 128:(k + 1) * 128, js])
                            xbk = kp.tile([128, NC], BF16, tag=f"xb{k}")
                            nc.gpsimd.tensor_copy(out=xbk[:], in_=xf[:])
                            xb.append(xbk)
                            sq = kp.tile([128, NC], BF16, tag="sq")
                            nc.scalar.activation(sq[:], xbk[:], AF.Square)
                            nc.tensor.matmul(x2p[:], lhsT=ones_col_b[:], rhs=sq[:],
                                             start=(k == 0), stop=(k == KT - 1))
                        # x2 output row + folded w row (bf16)
                        x2row = kp.tile([1, NC], F32, tag="x2row")
                        nc.scalar.copy(x2row[:], x2p[:])
                        nc.sync.dma_start(x2_o[0:1, js], x2row[:])
                        wrow = kp.tile([1, NC], BF16, tag="wrow")
                        nc.vector.tensor_scalar(
                            out=wrow[:], in0=x2p[:], scalar1=-0.5, scalar2=SHIFT,
                            op0=ALU.mult, op1=ALU.add)
                        for ct in range(CTN):
                            cxp = cps.tile([128, NC], F32, tag=f"cx{ct}")
                            for k in range(KT):
                                nc.tensor.matmul(
                                    cxp[:], lhsT=conb[k][:, ct * 128:(ct + 1) * 128],
                                    rhs=xb[k][:], start=(k == 0), stop=False)
                            nc.tensor.matmul(cxp[:], lhsT=ones_row_b[:], rhs=wrow[:],
                                             start=False, stop=True)
                            nc.scalar.copy(m[ct][:, js], cxp[:])

                # ---------------- extraction (DVE) + score (PE/ACT) ----------------
                with tc.tile_pool(name="sc", bufs=2) as sp, \
                     tc.tile_pool(name="spsum", bufs=2, space="PSUM") as sps:
                    # extraction of local top-KL per concept row
                    for ct in range(CTN):
                        mtop = sp.tile([128, KL], F32, tag="mtop")
                        itop = sp.tile([128, KL], U32, tag="itop")
                        for r in range(KL // 8):
                            mx8 = sp.tile([128, 8], BF16, tag="mx8")
                            ix8 = sp.tile([128, 8], U32, tag="ix8")
                            nc.vector.max(out=mx8[:], in_=m[ct][:])
                            nc.vector.max_index(out=ix8[:], in_max=mx8[:],
                                                in_values=m[ct][:])
                            nc.vector.tensor_copy(out=mtop[:, r * 8:(r + 1) * 8],
                                                  in_=mx8[:])
                            nc.vector.tensor_copy(out=itop[:, r * 8:(r + 1) * 8],
                                                  in_=ix8[:])
                            if r < KL // 8 - 1:
                                nc.vector.match_replace(
                                    out=m[ct][:], in_to_replace=mx8[:],
                                    in_values=m[ct][:], imm_value=NEG)
                        nc.sync.dma_start(mtop_o[ct * 128:(ct + 1) * 128, :], mtop[:])
                        nc.sync.dma_start(itop_o[ct * 128:(ct + 1) * 128, :], itop[:])

                    # ---- score + prob (fp32) ----
                    conf = [sp.tile([128, C], F32, tag=f"conf{k}", name=f"conf{k}") for k in range(KT)]
                    for k in range(KT):
                        nc.sync.dma_start(conf[k][:], con[k * 128:(k + 1) * 128, :])
                    # normc: sum over D of con^2 via f32 ones matmul
                    ncp = sps.tile([1, C], F32, tag="ncp")
                    for k in range(KT):
                        csq = sp.tile([128, C], F32, tag="csq")
                        nc.vector.tensor_tensor(out=csq[:], in0=conf[k][:],
                                                in1=conf[k][:], op=ALU.mult)
                        nc.tensor.matmul(ncp[:], lhsT=ones_col_f[:], rhs=csq[:],
                                         start=(k == 0), stop=(k == KT - 1))
                    ncrow = sp.tile([1, C], F32, tag="ncrow")
                    nc.scalar.activation(ncrow[:], ncp[:], AF.Sqrt)
                    thr_row = sp.tile([1, C], F32, tag="thrrow")
                    nc.vector.tensor_scalar_mul(thr_row[:], ncrow[:], 0.1)
                    thrp = sps.tile([128, C], F32, tag="thrp")
                    nc.tensor.matmul(thrp[:], lhsT=ones_row_f[:], rhs=thr_row[:],
                                     start=True, stop=True)
                    nc.scalar.copy(thr128[:], thrp[:])

                    for bt in range(btn):
                        tef = sp.tile([128, D], F32, tag="tef")
                        nc.sync.dma_start(tef[:], te[bt * 128:(bt + 1) * 128, :])
                        # row norm
                        tsq = sp.tile([128, D], F32, tag="tsq")
                        nc.vector.tensor_tensor(out=tsq[:], in0=tef[:], in1=tef[:],
                                                op=ALU.mult)
                        nt2 = sp.tile([128, 1], F32, tag="nt2")
                        nc.vector.reduce_sum(out=nt2[:], in_=tsq[:], axis=AX.X)
                        nc.scalar.activation(nte[:, bt:bt + 1], nt2[:], AF.Sqrt)
                        # transpose te tile -> teT (lhsT for score)
                        teT = []
                        for k in range(KT):
                            tp = sps.tile([128, 128], F32, tag="tp")
                            nc.tensor.transpose(tp[:], tef[:, k * 128:(k + 1) * 128],
                                                ident[:])
                            tt = sp.tile([128, 128], F32, tag=f"teT{k}")
                            nc.scalar.copy(tt[:], tp[:])
                            teT.append(tt)
                        scp = sps.tile([128, C], F32, tag="scp")
                        for k in range(KT):
                            nc.tensor.matmul(scp[:], lhsT=teT[k][:], rhs=conf[k][:],
                                             start=(k == 0), stop=(k == KT - 1))
                        thr_bt = sp.tile([128, C], F32, tag="thrbt")
                        nc.vector.tensor_scalar(
                            out=thr_bt[:], in0=thr128[:], scalar1=nte[:, bt:bt + 1],
                            scalar2=None, op0=ALU.mult)
                        mk = sp.tile([128, C], F32, tag="mk")
                        nc.vector.tensor_tensor(out=mk[:], in0=scp[:], in1=thr_bt[:],
                                                op=ALU.is_gt)
                        ts = sp.tile([128, C], F32, tag="ts")
                        nc.vector.tensor_tensor(out=ts[:], in0=scp[:], in1=mk[:],
                                                op=ALU.mult)
                        rs = sp.tile([128, 1], F32, tag="rs")
                        nc.vector.reduce_sum(out=rs[:], in_=ts[:], axis=AX.X)
                        nc.vector.tensor_scalar_add(rs[:], rs[:], 0.001)
                        rr = sp.tile([128, 1], F32, tag="rr")
                        nc.vector.reciprocal(rr[:], rs[:])
                        pr = sp.tile([128, C], F32, tag="pr")
                        nc.vector.tensor_scalar(
                            out=pr[:], in0=ts[:], scalar1=rr[:], scalar2=None,
                            op0=ALU.mult)
                        nc.sync.dma_start(prob_o[bt * 128:(bt + 1) * 128, :], pr[:])

            # ---------------- MLP + gram (fp32r) ----------------
            with tc.tile_pool(name="mp", bufs=2) as mp, \
                 tc.tile_pool(name="mpsum", bufs=2, space="PSUM") as mps:
                r1r = [mp.tile([128, H], F32R, tag=f"r1r{ct}", name=f"r1r{ct}") for ct in range(CTN)]
                for ct in range(CTN):
                    r1f = mp.tile([128, H], F32, tag="r1f")
                    nc.sync.dma_start(r1f[:], r1[ct * 128:(ct + 1) * 128, :])
                    nc.vector.tensor_copy(out=r1r[ct][:], in_=r1f[:])
                r2r = [mp.tile([128, D], F32R, tag=f"r2r{k}", name=f"r2r{k}") for k in range(KT)]
                for k in range(KT):
                    r2f = mp.tile([128, D], F32, tag="r2f")
                    nc.sync.dma_start(r2f[:], r2[k * 128:(k + 1) * 128, :])
                    nc.vector.tensor_copy(out=r2r[k][:], in_=r2f[:])

                identr = mp.tile([128, 128], F32R, tag="identr")
                nc.vector.tensor_copy(out=identr[:], in_=ident[:])

                for bt in range(btn):
                    prf = mp.tile([128, C], F32, tag="prf")
                    nc.sync.dma_start(prf[:], prob_o[bt * 128:(bt + 1) * 128, :])
                    probT = []
                    for ct in range(CTN):
                        pp = mps.tile([128, 128], F32, tag="tpm")
                        nc.tensor.transpose(pp[:], prf[:, ct * 128:(ct + 1) * 128],
                                            ident[:])
                        pt = mp.tile([128, 128], F32R, tag=f"probT{ct}")
                        nc.scalar.copy(pt[:], pp[:])
                        probT.append(pt)
                    hsb = mp.tile([128, H], F32, tag="hsb")
                    for hc in range(H // 512):
                        hp = mps.tile([128, 512], F32, tag="hp")
                        for ct in range(CTN):
                            nc.tensor.matmul(
                                hp[:], lhsT=probT[ct][:],
                                rhs=r1r[ct][:, hc * 512:(hc + 1) * 512],
                                start=(ct == 0), stop=(ct == CTN - 1))
                        nc.scalar.activation(hsb[:, hc * 512:(hc + 1) * 512], hp[:],
                                             AF.Relu)
                    hT = []
                    for k in range(KT):
                        tp2 = mps.tile([128, 128], F32, tag="tpm")
                        nc.tensor.transpose(tp2[:], hsb[:, k * 128:(k + 1) * 128],
                                            ident[:])
                        ht = mp.tile([128, 128], F32R, tag=f"hT{k}")
                        nc.scalar.copy(ht[:], tp2[:])
                        hT.append(ht)
                    recsb = mp.tile([128, D], F32, tag="recsb")
                    for dc in range(D // 512):
                        rp = mps.tile([128, 512], F32, tag="rp")
                        for k in range(KT):
                            nc.tensor.matmul(
                                rp[:], lhsT=hT[k][:],
                                rhs=r2r[k][:, dc * 512:(dc + 1) * 512],
                                start=(k == 0), stop=(k == KT - 1))
                        nc.scalar.copy(recsb[:, dc * 512:(dc + 1) * 512], rp[:])
                    nc.sync.dma_start(rec_o[bt * 128:(bt + 1) * 128, :], recsb[:])

                # gram off-diagonal sum of squares
                conr = [mp.tile([128, C], F32R, tag=f"conr{k}", name=f"conr{k}") for k in range(KT)]
                for k in range(KT):
                    cf2 = mp.tile([128, C], F32, tag="cf2")
                    nc.sync.dma_start(cf2[:], con[k * 128:(k + 1) * 128, :])
                    nc.vector.tensor_copy(out=conr[k][:], in_=cf2[:])
                gacc = mp.tile([128, CTN], F32, tag="gacc")
                for ct in range(CTN):
                    gp = mps.tile([128, C], F32, tag="hp")
                    for k in range(KT):
                        nc.tensor.matmul(gp[:],
                                         lhsT=conr[k][:, ct * 128:(ct + 1) * 128],
                                         rhs=conr[k][:], start=(k == 0),
                                         stop=(k == KT - 1))
                    gsb = mp.tile([128, C], F32, tag="gsb")
                    nc.scalar.copy(gsb[:], gp[:])
                    goffd = mp.tile([128, C], F32, tag="goffd")
                    nc.gpsimd.affine_select(
                        out=goffd[:], in_=gsb[:], pattern=[[1, C]],
                        compare_op=ALU.not_equal, fill=0.0,
                        base=-ct * 128, channel_multiplier=-1)
                    gsq = mp.tile([128, C], F32, tag="gsq")
                    nc.vector.tensor_tensor(out=gsq[:], in0=goffd[:], in1=goffd[:],
                                            op=ALU.mult)
                    nc.vector.reduce_sum(out=gacc[:, ct:ct + 1], in_=gsq[:], axis=AX.X)
                nc.sync.dma_start(goff_o[:, :], gacc[:])

    nc.compile()
    return nc


_CACHE = {}


def _get_nc():
    if "nc" not in _CACHE:
        _CACHE["nc"] = build()
    return _CACHE["nc"]


def kernel(train_embedding, sampled_train_embeddings, concept,
           rec_vector_1, rec_vector_2, topk):
    assert int(topk) == 64
    te = np.ascontiguousarray(np.asarray(train_embedding, dtype=np.float32))
    X = np.ascontiguousarray(np.asarray(sampled_train_embeddings, dtype=np.float32))
    con = np.ascontiguousarray(np.asarray(concept, dtype=np.float32))
    r1 = np.ascontiguousarray(np.asarray(rec_vector_1, dtype=np.float32))
    r2 = np.ascontiguousarray(np.asarray(rec_vector_2, dtype=np.float32))

    nc = _get_nc()
    in_maps = []
    for c in range(NCORES):
        in_maps.append({
            "te": te[c * BS:(c + 1) * BS],
            "xs": X[:, c * NS:(c + 1) * NS],
            "con": con, "r1": r1, "r2": r2,
        })
    res = run_bass_kernel_spmd(nc, in_maps, list(range(NCORES))).results

    rec = np.concatenate([res[c]["rec"] for c in range(NCORES)], axis=0)
    prob = np.concatenate([res[c]["prob"] for c in range(NCORES)], axis=0)

    # merge local top-k -> global top-64 mean of cx
    m_all, cx_all = [], []
    for c in range(NCORES):
        mv = res[c]["mtop"].astype(np.float32)            # shifted metric values
        iv = res[c]["itop"].astype(np.int64)              # local column idx
        x2 = res[c]["x2o"][0].astype(np.float32)          # [NS]
        # reconstruct the exact bf16 w-row the device folded into the metric,
        # so the returned value decodes without w-quantization noise
        w_dev = (-0.5 * x2 + SHIFT).astype(ml_dtypes.bfloat16).astype(np.float32)
        cx = mv - w_dev[iv]
        m_all.append(mv)
        cx_all.append(cx)
    M = np.concatenate(m_all, axis=1)
    CX = np.concatenate(cx_all, axis=1)
    sel = np.argsort(-M, axis=1)[:, :64]
    L1 = np.float32(np.take_along_axis(CX, sel, axis=1).mean(dtype=np.float64))

    goff = res[0]["goff"].astype(np.float64)
    L2 = np.float32(goff.sum() / (C * (C - 1)))

    return rec, prob, L1, L2
